# revision 6
# baseline (speedup 1.0000x reference)
"""Trainium2 Bass kernel for a GQA causal-attention block (TP over heads, 8 cores).

Computation (per reference): q/k/v projections of x, interleaved RoPE on q/k,
GQA causal attention (32 q heads, 8 kv heads, head_dim 128, seq 2048), output
projection. Sharding: tensor-parallel over heads — each core owns 4 q heads and
their shared kv head. The attention output (transposed layout) is AllGathered
across cores per 512-sequence window and each core computes a 512-column slice
of the final output projection; the host concatenates the column slices.

Device dataflow notes:
  - Everything transposed: x^T streams as the matmul moving operand so q^T/k^T
    come out with head_dim on partitions; scores are computed transposed
    (s^T[k_pos, q_pos]) so exp(s^T) feeds the PV matmul directly as the moving
    operand without any on-chip transposes.
  - RoPE uses a half-split head_dim permutation (evens then odds), folded into
    the wq/wk columns on the host, so the rotation is two 64-partition
    multiply/add pairs against host-transposed cos/sin tables.
  - Softmax skips the max subtraction (scores ~ N(0,1) after scaling); row sums
    come from a ones-matmul accumulated alongside PV; normalization is folded
    into the psum->sbuf eviction of the attention output via a PE-broadcast
    reciprocal row.
  - Causal masking multiplies exp(scores) by a 0/1 mask on diagonal blocks only.
  - The AllGather is split per 512-seq window and each window's slice of the
    output projection is emitted right after it, so collectives and the wo
    matmuls overlap the next window's attention on the PE.
"""

import numpy as np
import ml_dtypes

import concourse.bass as bass
import concourse.mybir as mybir
import concourse.tile as tile
from concourse import bacc
from concourse.bass_utils import run_bass_kernel_spmd

N_CORES = 8
P = 128
SEQ = 2048
DIM = 4096
N_HEADS = 32
N_KV_HEADS = 8
HD = 128
QH = N_HEADS // N_CORES        # q heads per core
KD = DIM // P                  # contraction chunks
KG = 4                         # k-chunk DMA groups
KPG = KD // KG                 # k chunks per group
W = 512                        # seq window (matmul moving free dim)
NW = SEQ // W
NT = SEQ // P
OUTC = DIM // N_CORES          # output columns per core
SCALE = HD ** -0.5

BF16 = mybir.dt.bfloat16
F32 = mybir.dt.float32


def _build_nc():
    nc = bacc.Bacc("TRN2", target_bir_lowering=False, debug=False,
                   num_devices=N_CORES)

    xt_d = nc.dram_tensor("xt", [DIM, SEQ], BF16, kind="ExternalInput")
    wq_d = nc.dram_tensor("wq", [DIM, QH * HD], BF16, kind="ExternalInput")
    wk_d = nc.dram_tensor("wk", [DIM, HD], BF16, kind="ExternalInput")
    wv_d = nc.dram_tensor("wv", [DIM, HD], BF16, kind="ExternalInput")
    wo_d = nc.dram_tensor("wo", [DIM, OUTC], BF16, kind="ExternalInput")
    cs_d = nc.dram_tensor("cs", [64, SEQ], F32, kind="ExternalInput")
    sn_d = nc.dram_tensor("sn", [64, SEQ], F32, kind="ExternalInput")
    out_d = nc.dram_tensor("out", [SEQ, OUTC], F32, kind="ExternalOutput")

    # 0/1 causal masks for the 4 diagonal alignments of a [128 kv, 512 q] block:
    # mask[p, c, q] = 1 iff kv offset p + c*128 <= q (within the 512-q window).
    j = np.arange(P)[:, None, None]
    c = np.arange(4)[None, :, None]
    q = np.arange(W)[None, None, :]
    masks_np = (j + c * P <= q).astype(ml_dtypes.bfloat16)
    masks_d = nc.inline_tensor(masks_np, "cmasks")
    ones_d = nc.inline_tensor(np.ones([P, P], dtype=ml_dtypes.bfloat16), "ones")
    ones1f_d = nc.inline_tensor(np.ones([1, P], dtype=np.float32), "ones1f")

    Exp = mybir.ActivationFunctionType.Exp

    with tile.TileContext(nc) as tc:
        with tc.tile_pool(name="const", bufs=1) as constp, \
             tc.tile_pool(name="acts", bufs=1) as actp:
            qrot = actp.tile([P, QH, SEQ], BF16)   # q^T (rope'd), per head
            krot = actp.tile([P, SEQ], BF16)       # k^T (rope'd)
            v_sb = actp.tile([P, NT, HD], BF16)    # v row-tiles [seq, hd]

            # ---- Phase 1: QKV projections + RoPE ----
            with tc.tile_pool(name="wqkv", bufs=1) as wp, \
                 tc.tile_pool(name="xtp", bufs=2) as xp, \
                 tc.tile_pool(name="psq", bufs=2, space="PSUM") as pq, \
                 tc.tile_pool(name="psv", bufs=2, space="PSUM") as pv, \
                 tc.tile_pool(name="ropet", bufs=2) as rtp:
                # interleave window-0 x^T and wq group loads so the first
                # matmuls start after ~2MB of DMA instead of ~10MB.
                wq_g = [wp.tile([P, KPG, QH * HD], BF16, name=f"wq{g}")
                        for g in range(KG)]
                xt_gs: dict[tuple[int, int], bass.AP] = {}

                def load_xt_group(w, g):
                    t = xp.tile([P, KPG, W], BF16, tag=f"xt{g}")
                    k0 = g * KPG
                    nc.sync.dma_start(
                        out=t[:],
                        in_=xt_d[k0 * P:(k0 + KPG) * P, w * W:(w + 1) * W]
                        .rearrange("(k p) n -> p k n", p=P))
                    xt_gs[(w, g)] = t

                for g in range(KG):
                    load_xt_group(0, g)
                    nc.sync.dma_start(
                        out=wq_g[g][:],
                        in_=wq_d[g * KPG * P:(g + 1) * KPG * P, :]
                        .rearrange("(k p) n -> p k n", p=P))
                    if g == 0:
                        cs_sb = constp.tile([64, SEQ], F32)
                        nc.sync.dma_start(out=cs_sb[:], in_=cs_d[:])
                        sn_sb = constp.tile([64, SEQ], F32)
                        nc.sync.dma_start(out=sn_sb[:], in_=sn_d[:])
                wk_sb = wp.tile([P, KD, HD], BF16)
                nc.sync.dma_start(out=wk_sb[:],
                                  in_=wk_d[:].rearrange("(k p) n -> p k n", p=P))
                wv_sb = wp.tile([P, KD, HD], BF16)
                nc.sync.dma_start(out=wv_sb[:],
                                  in_=wv_d[:].rearrange("(k p) n -> p k n", p=P))
                masks_sb = constp.tile([P, 4, W], BF16)
                nc.sync.dma_start(out=masks_sb[:], in_=masks_d[:])
                ones_sb = constp.tile([P, P], BF16)
                nc.sync.dma_start(out=ones_sb[:], in_=ones_d[:])
                ones1f_sb = constp.tile([1, P], F32)
                nc.sync.dma_start(out=ones1f_sb[:], in_=ones1f_d[:])

                for w in range(NW):
                    s0 = w * W
                    if w + 1 < NW:
                        for g in range(KG):
                            load_xt_group(w + 1, g)
                    cw = cs_sb[:, s0:s0 + W]
                    sw = sn_sb[:, s0:s0 + W]
                    for m in range(QH + 1):
                        ps = pq.tile([P, W], F32, tag="psq")
                        for k in range(KD):
                            lhsT = (wq_g[k // KPG][:, k % KPG, m * HD:(m + 1) * HD]
                                    if m < QH else wk_sb[:, k, :])
                            nc.tensor.matmul(ps[:], lhsT,
                                             xt_gs[(w, k // KPG)][:, k % KPG, :],
                                             start=(k == 0), stop=(k == KD - 1))
                        dst = (qrot[:, m, s0:s0 + W] if m < QH
                               else krot[:, s0:s0 + W])
                        t1 = rtp.tile([64, W], F32, tag="t1")
                        t2 = rtp.tile([64, W], F32, tag="t2")
                        # even half: x1*cos - x2*sin
                        nc.vector.tensor_mul(t1[:], ps[0:64, :], cw)
                        nc.vector.tensor_mul(t2[:], ps[64:128, :], sw)
                        nc.vector.tensor_sub(dst[0:64], t1[:], t2[:])
                        # odd half: x1*sin + x2*cos
                        t3 = rtp.tile([64, W], F32, tag="t3")
                        t4 = rtp.tile([64, W], F32, tag="t4")
                        nc.vector.tensor_mul(t3[:], ps[0:64, :], sw)
                        nc.vector.tensor_mul(t4[:], ps[64:128, :], cw)
                        nc.vector.tensor_add(dst[64:128], t3[:], t4[:])

                    for mm in range(W // P):
                        psv = pv.tile([P, HD], F32, tag="psv")
                        for k in range(KD):
                            nc.tensor.matmul(
                                psv[:],
                                xt_gs[(w, k // KPG)][:, k % KPG, mm * P:(mm + 1) * P],
                                wv_sb[:, k, :],
                                start=(k == 0), stop=(k == KD - 1))
                        nc.scalar.copy(v_sb[:, w * (W // P) + mm, :], psv[:])

            # ---- Phase 2+3: attention, windowed AllGather, output proj ----
            with tc.tile_pool(name="ccdram", bufs=1, space="DRAM") as dcc, \
                 tc.tile_pool(name="wop", bufs=1) as wop, \
                 tc.tile_pool(name="pss", bufs=2, space="PSUM") as psp, \
                 tc.tile_pool(name="pso", bufs=2, space="PSUM") as pso, \
                 tc.tile_pool(name="pssum", bufs=2, space="PSUM") as pss, \
                 tc.tile_pool(name="psw", bufs=2, space="PSUM") as psw, \
                 tc.tile_pool(name="ptp", bufs=4) as ptp, \
                 tc.tile_pool(name="attp", bufs=2) as attp, \
                 tc.tile_pool(name="atfp", bufs=3) as atfp, \
                 tc.tile_pool(name="outp", bufs=2) as outp:
                wo_sb = wop.tile([P, KD, OUTC], BF16)
                nc.sync.dma_start(out=wo_sb[:],
                                  in_=wo_d[:].rearrange("(k p) n -> p k n", p=P))
                ag_in = [dcc.tile([QH * HD, W], BF16, name=f"agin{w}")
                         for w in range(NW)]
                ag_out = [dcc.tile([DIM, W], BF16, addr_space="Shared",
                                   name=f"agout{w}")
                          for w in range(NW)]

                for qc in range(NW):
                    q0 = qc * W
                    nkv = (W // P) * (qc + 1)
                    for h in range(QH):
                        ps_o = pso.tile([P, W], F32, tag="o")
                        ps_sum = pss.tile([P, W], F32, tag="sum")
                        for jj in range(nkv):
                            j0 = jj * P
                            ps_s = psp.tile([P, W], F32, tag="s")
                            nc.tensor.matmul(ps_s[:], krot[:, j0:j0 + P],
                                             qrot[:, h, q0:q0 + W],
                                             start=True, stop=True)
                            pt = ptp.tile([P, W], BF16, tag="pt")
                            nc.scalar.activation(pt[:], ps_s[:], Exp, scale=SCALE)
                            if jj >= (W // P) * qc:
                                nc.vector.tensor_mul(
                                    pt[:], pt[:],
                                    masks_sb[:, jj - (W // P) * qc, :])
                            nc.tensor.matmul(ps_o[:], v_sb[:, jj, :], pt[:],
                                             start=(jj == 0), stop=(jj == nkv - 1))
                            nc.tensor.matmul(ps_sum[:], ones_sb[:], pt[:],
                                             start=(jj == 0), stop=(jj == nkv - 1))
                        # normalization: reciprocal of one row, PE-broadcast to
                        # 128 partitions, multiply into the psum eviction.
                        inv1 = attp.tile([1, W], F32, tag="inv1")
                        nc.vector.reciprocal(inv1[:], ps_sum[0:1, :])
                        ps_bc = psp.tile([P, W], F32, tag="s")
                        nc.tensor.matmul(ps_bc[:], ones1f_sb[:], inv1[:],
                                         start=True, stop=True)
                        inv_bc = attp.tile([P, W], F32, tag="invbc")
                        nc.vector.tensor_copy(inv_bc[:], ps_bc[:])
                        at = attp.tile([P, W], BF16, tag="at")
                        nc.vector.tensor_mul(at[:], ps_o[:], inv_bc[:])
                        nc.sync.dma_start(out=ag_in[qc][h * HD:(h + 1) * HD, :],
                                          in_=at[:])

                    nc.gpsimd.collective_compute(
                        "AllGather", mybir.AluOpType.bypass,
                        replica_groups=[list(range(N_CORES))],
                        ins=[ag_in[qc][:]], outs=[ag_out[qc][:]])

                    # output projection for this sequence window
                    for mm in range(W // P):
                        m0 = mm * P
                        atf = atfp.tile([P, KD, P], BF16, tag="atf")
                        nc.sync.dma_start(
                            out=atf[:],
                            in_=ag_out[qc][:, m0:m0 + P]
                            .rearrange("(k p) n -> p k n", p=P))
                        ps = psw.tile([P, OUTC], F32, tag="w")
                        for k in range(KD):
                            nc.tensor.matmul(ps[:], atf[:, k, :], wo_sb[:, k, :],
                                             start=(k == 0), stop=(k == KD - 1))
                        ot = outp.tile([P, OUTC], F32, tag="ot")
                        nc.vector.tensor_copy(ot[:], ps[:])
                        nc.sync.dma_start(out=out_d[q0 + m0:q0 + m0 + P, :],
                                          in_=ot[:])

    nc.compile()
    return nc


_NC_CACHE = None


def _get_nc():
    global _NC_CACHE
    if _NC_CACHE is None:
        _NC_CACHE = _build_nc()
    return _NC_CACHE


def make_in_maps(x, freqs_cos, freqs_sin, wq, wk, wv, wo):
    bf16 = ml_dtypes.bfloat16
    # half-split permutation: evens then odds within each head's 128 dims
    pidx = np.concatenate([np.arange(0, HD, 2), np.arange(1, HD, 2)])
    xt = np.ascontiguousarray(np.asarray(x).T.astype(bf16))
    cs = np.ascontiguousarray(np.asarray(freqs_cos).T.astype(np.float32))
    sn = np.ascontiguousarray(np.asarray(freqs_sin).T.astype(np.float32))
    wq = np.asarray(wq)
    wk = np.asarray(wk)
    wv = np.asarray(wv)
    wo = np.asarray(wo)
    in_maps = []
    for core in range(N_CORES):
        q_cols = np.concatenate([h * HD + pidx
                                 for h in range(QH * core, QH * (core + 1))])
        in_maps.append({
            "xt": xt,
            "wq": np.ascontiguousarray(wq[:, q_cols].astype(bf16)),
            "wk": np.ascontiguousarray(wk[:, core * HD + pidx].astype(bf16)),
            "wv": np.ascontiguousarray(wv[:, core * HD:(core + 1) * HD].astype(bf16)),
            "wo": np.ascontiguousarray(wo[:, core * OUTC:(core + 1) * OUTC].astype(bf16)),
            "cs": cs,
            "sn": sn,
        })
    return in_maps


def kernel(x, freqs_cos, freqs_sin, wq, wk, wv, wo, _run_kwargs=None):
    in_maps = make_in_maps(x, freqs_cos, freqs_sin, wq, wk, wv, wo)
    nc = _get_nc()
    res = run_bass_kernel_spmd(nc, in_maps, list(range(N_CORES)),
                               **(_run_kwargs or {}))
    out = np.concatenate([res.results[i]["out"] for i in range(N_CORES)], axis=1)
    if _run_kwargs is not None:
        kernel.last_results = res
    return np.ascontiguousarray(out.astype(np.float32))
